# revision 77
# baseline (speedup 1.0000x reference)
"""Additive (Bahdanau) attention scoring kernel for Trainium2, 8-core SPMD.

Reference computation (B=16, S=4096, D=1024, all fp32):
    q      = target @ Wq.T                    # [B, D]
    k      = memory @ Wk.T                    # [B, S, D]
    scores = tanh(q[:, None, :] + k) @ v      # [B, S]
    out    = softmax(scores - 1e9 * mask, axis=-1)

Sharding: batch across the 8 cores (2 batches per core), weights replicated.

Host-side prep (layout + dtype only): memory is transposed to [D, S] per
batch, its columns compacted to just the unmasked positions (masked columns
contribute exactly 0 to the reference softmax: exp(-1e9) == 0 in fp32, so
skipping them is algebraically exact), padded with duplicates of the first
kept column to a 128-multiple, and cast to bf16 in an s-tile-blocked layout
(col = st*DC*P + dc*P + j) so each s-tile is one contiguous 256 KB DMA and
becomes compute-ready independently. Pad positions get a -1e4 exp bias on
device, so their exp is exactly 0 and the softmax denominator is exact.

Per-core device pipeline (python-unrolled, Tile-scheduled), with s on the
PSUM partition dim so the v-contraction runs on the DVE, not the PE:
  - k s-tiles [s=128, e=1024] with the memory chunk as the [128,128]
    stationary and Wk^T rows as the 512-wide moving operand, bf16 operands
    accumulated over the 8 d-chunks in fp32 PSUM (two bank-aligned e-halves
    per tile; matmul PSUM outputs must be fp32 and within one 2KB bank).
    Weight loads hide fully under the matmuls.
  - q via bf16 matmuls (target as the M=2 stationary), emitted after
    k-tile 0 so the PE starts on whichever DMA lands first; q rows are
    broadcast across partitions with a K=128 selector matmul (selectors
    shipped from host) into q_bc [128, NB*1024].
  - Per tile: DVE adds q_bc (scalar_tensor_tensor, PSUM in), ACT applies
    tanh (bf16 out), DVE multiplies by v and reduces along e in one
    scalar_tensor_tensor with fused accum_out -> score [128, 1]; ACT exp
    with the pad bias as per-partition bias writes one e_all column. The
    last tile's chain is split into e-halves to shorten the kernel tail.
  - Per-batch finale (deferred 2 tiles past the batch boundary): DVE
    reduce of e_all, K=128 ones-matmul partition-reduce, reciprocal,
    per-partition scale, DMA out. The host scatters the compact normalized
    rows to their full-S positions (masked positions are exactly 0).

NOTE: nc.vector.tensor_tensor_reduce and nc.gpsimd.scalar_tensor_tensor
(any GpSimd ALU compute) hard-faulted the device
(NRT_EXEC_UNIT_UNRECOVERABLE) despite passing CoreSim; matmuls
accumulating onto ACT-preloaded PSUM (start=False) ran but produced wrong
results on HW; matmul output dtype must be fp32 (bank limit 512 cols).
Avoid all of these.

Tried and measured slower-or-neutral on HW: fp8 in any viable split
(accuracy gate), eh-major wk layout, per-strip instead of per-s-tile DMAs,
deferring early-tile epilogues behind ACT PSUM-spill copies (startup gain
exactly cancelled by the DVE epilogue backlog), quarter-split last-tile
chain, mem prefetch depth != 3, batch pairing by tile count (9 of 16
batches need 17 s-tiles so the max core keeps 34 either way).
"""

from contextlib import ExitStack

import numpy as np
import ml_dtypes

import concourse.tile as tile
from concourse import bacc, mybir
import concourse.bass as bass  # noqa: F401

B, S, D = 16, 4096, 1024
N_CORES = 8
NB = B // N_CORES  # batches per core
P = 128
DC = D // P        # contraction chunks
SW = 512           # full strip width along compacted s (DMA granularity)

F32 = mybir.dt.float32
BF16 = mybir.dt.bfloat16
AF = mybir.ActivationFunctionType
ALU = mybir.AluOpType

_CACHE = {}


def pad_s(max_kept):
    """Compacted-s padded to a 128 multiple."""
    return max(P, ((max_kept + P - 1) // P) * P)


def _build_program(s_pad):
    n_st = s_pad // P  # s-tiles per batch

    nc = bacc.Bacc("TRN2", target_bir_lowering=False, debug=False)

    # s-tile-blocked: column index = st*DC*P + dc*P + j
    memC = nc.dram_tensor("memC", [NB, P, DC * s_pad], BF16, kind="ExternalInput").ap()
    wkL = nc.dram_tensor("wkL", [P, DC * D], BF16, kind="ExternalInput").ap()
    wqL = nc.dram_tensor("wqL", [P, DC * D], BF16, kind="ExternalInput").ap()
    tgtL = nc.dram_tensor("tgtL", [P, DC * NB], BF16, kind="ExternalInput").ap()
    vB = nc.dram_tensor("vB", [P, D], BF16, kind="ExternalInput").ap()
    pb2 = nc.dram_tensor("pb2", [NB, P, n_st], F32, kind="ExternalInput").ap()
    selC = nc.dram_tensor("selC", [P, NB * P], BF16, kind="ExternalInput").ap()
    out = nc.dram_tensor("out", [NB, P, n_st], F32, kind="ExternalOutput").ap()

    with tile.TileContext(nc) as tc, ExitStack() as ctx:
        consts = ctx.enter_context(tc.tile_pool(name="consts", bufs=1))
        mem_pool = ctx.enter_context(tc.tile_pool(name="mem", bufs=3))
        th_pool = ctx.enter_context(tc.tile_pool(name="th", bufs=3))
        sc_pool = ctx.enter_context(tc.tile_pool(name="scrap", bufs=2))
        fin_pool = ctx.enter_context(tc.tile_pool(name="fin", bufs=2))
        os_pool = ctx.enter_context(tc.tile_pool(name="os", bufs=2, space="PSUM"))
        qbc_pool = ctx.enter_context(tc.tile_pool(name="qbc", bufs=1, space="PSUM"))
        sm_pool = ctx.enter_context(tc.tile_pool(name="smps", bufs=2, space="PSUM"))

        # --- weights / small constants. wq first (the q matmuls are first in
        # PE order), wk after, small constants on the Vector issue queue so
        # they don't serialize behind the weights.
        # first k-tile's memory block leads the sync queue (HWDGE): it plus
        # wq/wk chunk 0 are all the PE needs to start
        mem0_sb = mem_pool.tile([P, DC * P], BF16, tag="mem", name="mem_sb")
        nc.sync.dma_start(mem0_sb[:], memC[0][:, 0:DC * P])
        wq_sb = consts.tile([P, DC * D], BF16)
        for c in range(DC):
            nc.sync.dma_start(
                wq_sb[:, c * D:(c + 1) * D], wqL[:, c * D:(c + 1) * D]
            )
        wk_sb = consts.tile([P, DC * D], BF16)
        for c in range(DC):
            nc.sync.dma_start(
                wk_sb[:, c * D:(c + 1) * D], wkL[:, c * D:(c + 1) * D]
            )
        # mem tiles 1-2 lead the ACT HWDGE queue: the small constants after
        # them all have several us of slack before first use
        mem12_sb = []
        for st in (1, 2):
            m = mem_pool.tile([P, DC * P], BF16, tag="mem", name="mem_sb")
            nc.scalar.dma_start(m[:], memC[0][:, st * DC * P:(st + 1) * DC * P])
            mem12_sb.append(m)
        tgt_sb = consts.tile([P, DC * NB], BF16)
        nc.scalar.dma_start(tgt_sb[:], tgtL[:, :])
        v_bc = consts.tile([P, D], BF16)
        nc.scalar.dma_start(v_bc[:], vB[:, :])
        pb_sb = consts.tile([P, NB * n_st], F32)
        for b in range(NB):
            nc.scalar.dma_start(pb_sb[:, b * n_st:(b + 1) * n_st], pb2[b])

        ones_sq = consts.tile([P, P], F32)
        nc.vector.memset(ones_sq[:], 1.0)
        # selector blocks (from host): selC[:, b*P:(b+1)*P] is 1 in row b, 0
        # elsewhere -> K=128 matmul broadcasts q row b across 128 partitions
        sel_sb = consts.tile([P, NB * P], BF16)
        nc.scalar.dma_start(sel_sb[:], selC[:, :])

        q_bc = consts.tile([P, NB * D], BF16)
        q_row = consts.tile([NB, D], F32)
        q_pad = consts.tile([P, D], BF16)
        nc.vector.memset(q_pad[:], 0.0)

        # PE warm-up: dummy matmuls on memset operands fill the otherwise
        # idle DMA-wait window at kernel start so the DVFS clock ramps
        # before the real k-stream and q matmuls arrive
        warm_st = consts.tile([P, P], BF16)
        nc.vector.memset(warm_st[:], 0.01)
        warm_mv = consts.tile([P, SW], BF16)
        nc.vector.memset(warm_mv[:], 0.01)
        warm_ps = qbc_pool.tile([P, SW], F32, tag="qbc", name="warm_ps")
        for w in range(8):
            nc.tensor.matmul(
                warm_ps[:], warm_st[:], warm_mv[:],
                start=(w == 0), stop=(w == 7),
            )

        def emit_q_path():
            # q[b, e] = sum_d target[b, d] * Wq[e, d]: target as the M=2
            # stationary, Wq^T as the N=512 moving operand. Emitted AFTER
            # k-tile 0's matmuls so the PE starts on whichever operands land
            # first; q_bc is only needed by tile 0's DVE q-add, ~2 tiles in.
            for j in range(D // SW):
                q_ps = sm_pool.tile([NB, SW], F32, tag="qps", name="q_ps")
                for dc in range(DC):
                    nc.tensor.matmul(
                        q_ps[:],
                        tgt_sb[:, dc * NB:(dc + 1) * NB],
                        wq_sb[:, dc * D + j * SW: dc * D + (j + 1) * SW],
                        start=(dc == 0),
                        stop=(dc == DC - 1),
                    )
                nc.vector.tensor_copy(q_row[:, j * SW:(j + 1) * SW], q_ps[:])
            nc.vector.tensor_copy(q_pad[0:NB, :], q_row[:])
            # broadcast q rows along partitions: K=128 selector matmul
            for b in range(NB):
                qb_ps = qbc_pool.tile([P, D], F32, tag="qbc", name="qb_ps")
                for eh in range(2):
                    nc.tensor.matmul(
                        qb_ps[:, eh * SW:(eh + 1) * SW],
                        sel_sb[:, b * P:(b + 1) * P],
                        q_pad[:, eh * SW:(eh + 1) * SW],
                        start=True, stop=True,
                    )
                nc.vector.tensor_copy(q_bc[:, b * D:(b + 1) * D], qb_ps[:])

        # one 256 KB DMA per s-tile, all issued up front on the (otherwise
        # idle) GpSimd queue; mem_pool bufs throttle them to a rolling
        # prefetch window and each tile becomes ready independently
        tiles = []  # (b, col, mem_sb)
        for b in range(NB):
            for st in range(n_st):
                if b == 0 and st == 0:
                    tiles.append((b, 0, mem0_sb))
                    continue
                if b == 0 and st in (1, 2):
                    tiles.append((b, st, mem12_sb[st - 1]))
                    continue
                mem_sb = mem_pool.tile([P, DC * P], BF16, tag="mem", name="mem_sb")
                nc.gpsimd.dma_start(
                    mem_sb[:], memC[b][:, st * DC * P:(st + 1) * DC * P]
                )
                tiles.append((b, b * n_st + st, mem_sb))

        pending_fin = []
        e_alls = {}

        def emit_epilogue(i, b, col, src_ap):
            ti = th_pool.tile([P, D], BF16, tag="ti", name="ti")
            th = th_pool.tile([P, D], BF16, tag="th", name="th")
            scrap = sc_pool.tile([P, D], BF16, tag="sc", name="scrap")
            sc_pre = sc_pool.tile([P, 2], F32, tag="scp", name="sc_pre")
            if i == len(tiles) - 1:
                # split the epilogue chain into e-halves so half of it
                # overlaps the second half's matmuls (shortens the kernel
                # tail, where nothing else hides this latency)
                for eh in range(2):
                    hs = slice(eh * SW, (eh + 1) * SW)
                    nc.scalar.activation(th[:, hs], src_ap[:, hs], AF.Tanh)
                    nc.vector.scalar_tensor_tensor(
                        scrap[:, hs], th[:, hs], 1.0, v_bc[:, hs],
                        ALU.mult, ALU.mult,
                        accum_out=sc_pre[:, eh:eh + 1],
                    )
                nc.vector.tensor_add(
                    sc_pre[:, 0:1], sc_pre[:, 0:1], sc_pre[:, 1:2]
                )
            else:
                nc.vector.scalar_tensor_tensor(
                    ti[:], src_ap[:], 1.0, q_bc[:, b * D:(b + 1) * D],
                    ALU.mult, ALU.add,
                )
                nc.scalar.activation(th[:], ti[:], AF.Tanh)
                nc.vector.scalar_tensor_tensor(
                    scrap[:], th[:], 1.0, v_bc[:], ALU.mult, ALU.mult,
                    accum_out=sc_pre[:, 0:1],
                )
            # exp with the pad bias folded in as the per-partition ACT bias
            if b not in e_alls:
                e_alls[b] = fin_pool.tile([P, n_st], F32, tag="eall", name="e_all")
            nc.scalar.activation(
                e_alls[b][:, col - b * n_st: col - b * n_st + 1],
                sc_pre[:, 0:1], AF.Exp, bias=pb_sb[:, col:col + 1],
            )
            if col == (b + 1) * n_st - 1:
                pending_fin.append(b)

        for i, (b, col, mem_sb) in enumerate(tiles):
            os_ps = os_pool.tile([P, D], F32, tag="os", name="os_ps")
            last = i == len(tiles) - 1
            for eh in range(2):
                for dc in range(DC):
                    nc.tensor.matmul(
                        os_ps[:, eh * SW:(eh + 1) * SW],
                        mem_sb[:, dc * P:(dc + 1) * P],
                        wk_sb[:, dc * D + eh * SW: dc * D + (eh + 1) * SW],
                        start=(dc == 0),
                        stop=(dc == DC - 1) and not last,
                    )
                if last:
                    # fold the q-add into the accumulation group: drops the
                    # serial DVE q-add from the kernel tail where nothing
                    # hides it (+0.43 us PE for -1.6 us tail)
                    nc.tensor.matmul(
                        os_ps[:, eh * SW:(eh + 1) * SW],
                        sel_sb[:, b * P:(b + 1) * P],
                        q_pad[:, eh * SW:(eh + 1) * SW],
                        start=False,
                        stop=True,
                    )
            if i == 0:
                emit_q_path()
            emit_epilogue(i, b, col, os_ps)

            # per-batch softmax finale, deferred 2 tiles past the batch
            # boundary so its tiny PE matmul never stalls the k-stream
            for fb in list(pending_fin):
                if i >= (fb + 1) * n_st - 1 + 2 or i == len(tiles) - 1:
                    pending_fin.remove(fb)
                    e_all = e_alls[fb]
                    esum = fin_pool.tile([P, 1], F32, tag="esum", name="esum")
                    nc.vector.reduce_sum(
                        esum[:], e_all[:], axis=mybir.AxisListType.X
                    )
                    tot_ps = sm_pool.tile([P, 1], F32, tag="qps", name="tot_ps")
                    nc.tensor.matmul(
                        tot_ps[:], ones_sq[:], esum[:], start=True, stop=True
                    )
                    rec = fin_pool.tile([P, 1], F32, tag="rec", name="rec")
                    nc.vector.reciprocal(rec[:], tot_ps[:])
                    out_n = fin_pool.tile([P, n_st], F32, tag="outn", name="out_n")
                    nc.vector.tensor_scalar_mul(out_n[:], e_all[:], rec[:, 0:1])
                    nc.sync.dma_start(out[fb], out_n[:])

    nc.compile()
    return nc


def get_program(s_pad=None):
    assert s_pad is not None
    if s_pad not in _CACHE:
        _CACHE[s_pad] = _build_program(s_pad)
    return _CACHE[s_pad]


def prepare_in_maps(memory, target, memory_mask, Wq, Wk, v):
    memory = np.asarray(memory, dtype=np.float32)
    target = np.asarray(target, dtype=np.float32)
    Wq = np.asarray(Wq, dtype=np.float32)
    Wk = np.asarray(Wk, dtype=np.float32)
    v = np.asarray(v, dtype=np.float32)
    mask = np.asarray(memory_mask)

    keep_bool = ~mask                                                # [B, S]
    n_kept = keep_bool.sum(1)
    s_pad = pad_s(int(n_kept.max()))
    n_st = s_pad // P

    memT = memory.transpose(0, 2, 1)                                 # [B, D, S] view
    kept_pad = np.empty((B, s_pad), dtype=np.int64)
    pad_bias = np.zeros((B, s_pad), dtype=np.float32)
    kept_lists = []
    for b in range(B):
        k = np.flatnonzero(keep_bool[b])
        kept_lists.append(k)
        kept_pad[b, :len(k)] = k
        kept_pad[b, len(k):] = k[0]  # pad data: duplicate first kept column
        pad_bias[b, len(k):] = -1e4  # pad scores -> exp == 0 exactly

    # compact position c = st*128 + p  ->  pb2[b, p, st]
    pb2 = np.ascontiguousarray(
        pad_bias.reshape(B, n_st, P).transpose(0, 2, 1))             # [B, P, n_st]

    # gather + [DC, P, n_st, P] -> s-tile-blocked [B, P, n_st*DC*P] bf16
    memC = np.empty((B, P, DC * s_pad), dtype=ml_dtypes.bfloat16)
    for b in range(B):
        g = memT[b][:, kept_pad[b]].reshape(DC, P, n_st, P)
        memC[b] = np.ascontiguousarray(
            g.transpose(1, 2, 0, 3)).reshape(P, n_st * DC * P)

    def wlayout(W):  # [P, DC*D]: col dc*D + e holds W[e, dc*128+p]
        return np.ascontiguousarray(
            W.T.reshape(DC, P, D).transpose(1, 0, 2).reshape(P, DC * D)
        ).astype(ml_dtypes.bfloat16)

    wkL = wlayout(Wk)
    wqL = wlayout(Wq)
    tgtL = np.ascontiguousarray(
        target.T.reshape(DC, P, B).transpose(1, 0, 2).reshape(P, DC * B)
    ).astype(ml_dtypes.bfloat16)                                     # [P, DC*B]
    vB = np.ascontiguousarray(
        np.broadcast_to(v.astype(ml_dtypes.bfloat16), (P, D)))       # [P, D]
    selC_h = np.zeros((P, NB * P), dtype=ml_dtypes.bfloat16)
    for b in range(NB):
        selC_h[b, b * P:(b + 1) * P] = 1

    in_maps = [
        {
            "memC": np.ascontiguousarray(memC[c * NB:(c + 1) * NB]),
            "wkL": wkL,
            "wqL": wqL,
            "tgtL": np.ascontiguousarray(
                tgtL.reshape(P, DC, B)[:, :, c * NB:(c + 1) * NB].reshape(P, DC * NB)
            ),
            "vB": vB,
            "pb2": np.ascontiguousarray(pb2[c * NB:(c + 1) * NB]),
            "selC": selC_h,
        }
        for c in range(N_CORES)
    ]
    return in_maps, s_pad, kept_lists


def gather_output(results, kept_lists):
    out = np.zeros((B, S), dtype=np.float32)
    for c in range(N_CORES):
        comp = results[c]["out"]                                     # [NB, P, n_st]
        for bl in range(NB):
            b = c * NB + bl
            k = kept_lists[b]
            vals = comp[bl].T.ravel()                                # c = st*128 + p
            out[b, k] = vals[:len(k)]
    return out


def kernel(memory, target, memory_mask, Wq, Wk, v):
    from concourse.bass_utils import run_bass_kernel_spmd

    in_maps, s_pad, kept_lists = prepare_in_maps(
        memory, target, memory_mask, Wq, Wk, v
    )
    nc = get_program(s_pad=s_pad)
    res = run_bass_kernel_spmd(nc, in_maps, list(range(N_CORES)))
    return gather_output(res.results, kept_lists)
